# revision 26
# baseline (speedup 1.0000x reference)
"""Trainium2 Bass kernel for nn_AveragedAdapter (dense_mlp).

Computes: loss = sum_{a,e} mean_{b,d} (gelu(f[:,a] @ W1[a,e] + b1[a,e]) @ W2[a,e]
                                        + b2[a,e] - target[:,a])^2 / E

Sharding: expert-parallel over the first expert axis `a` — core a computes the
full inner-e loop for its adapter row and returns a partial sum of squared
errors; the host adds the 8 partials and applies the 1/(B*D*E) scale.

The 16.8 MiB/core of fp8 weights dominate: the kernel is one long HBM->SBUF
stream over the 16 SDMA engines (~26.5 GB/s each), so exec time ~=
stream_start + per-engine bytes / rate + tail.  Design choices follow:

  - Weights fp8-e4m3 (loss is a mean over 33.5M squared errors; measured
    rel-err ~1e-3 end to end).  Biases and accumulation stay fp32.
  - W1[0] rides the scalar HWDGE ring (whose path is warmed by the ACT
    table static DMAs and starts ~2.5us before the sync ring's first
    byte); ft/b1 ride scalar ahead of it.  All other slabs stream on the
    sync ring in consumption order: W1[1..7], then W2 halves.
  - W2 slabs are split into halves so the last expert's first 4 matmuls
    start half a slab before the final byte lands.
  - The per-expert (target - b2) tensor is NOT streamed (it was 1 MiB of
    bf16).  Instead target[:,a] is loaded once as fp8 [128,512] and folded
    into the layer-2 PSUM accumulation with a -identity matmul; b2 is
    added with a [8,128] 0.125-ones x [8,512] 8x-replicated-b2 matmul.
    The Square+row-sum then reads PSUM directly (no DVE subtract, no err
    SBUF roundtrip) — the only DVE work left in phase 2 is the [128,1]
    running-sum chain.
  - Late-needed consts (tgt8, -I, b2) ride the gpsimd SWDGE ring so they
    never touch the HWDGE streams.
  - Throwaway matmuls warm the PE clock-gate during the first slab's
    flight (idle PE runs at 1.2 GHz; sustained activity unlocks 2.4 GHz).

Per-core program (a = core id):
  - phase 1 (all experts): layer 1 computes hT (H on partitions) with W1
    chunk-pairs stationary via fp8 DoubleRow ([128,2,128] x [128,2,128]),
    4 m-chunks per PSUM bank; bias add on DVE (broadcast over batch) -> bf16;
    exact-erf Gelu on ACT -> fp8 h kept in SBUF for all 8 experts.
  - phase 2 (all experts): layer 2 with h chunk-pairs stationary, W2 moving
    [128,2,512] DoubleRow, 8 matmuls + the -tgt and +b2 fix matmuls into one
    PSUM bank; Square+row-sum on ACT straight from PSUM (accum_out);
    [128,1] running sum chained on DVE; final cross-partition sum via a
    ones-vector matmul -> [1,1] -> DMA out.
"""

import sys

if "/opt/trn_rl_repo" not in sys.path:
    sys.path.insert(0, "/opt/trn_rl_repo")

import numpy as np
import ml_dtypes

B, E, D, M = 128, 8, 512, 4
H = M * D            # 2048
P = 128
KC1 = D // P         # 4  k-chunks in layer 1
MC = H // P          # 16 m-chunks of H / k-chunks in layer 2
NG = 4               # m-chunk groups (4 chunks -> one PSUM bank)
W1_COLS = KC1 * H    # 8192
W2_COLS = MC * D     # 8192
W2_HALF = W2_COLS // 2
F8 = ml_dtypes.float8_e4m3

_NC = None


def _build_nc(act="gelu"):
    import concourse.tile as tile
    from concourse import bacc, mybir

    act_fn = {
        "gelu": mybir.ActivationFunctionType.Gelu,
        "identity": mybir.ActivationFunctionType.Identity,
    }[act]
    # Bacc (not Bass): its compile() pass legalizes sync waits for the trn2
    # ISA's one-wait-per-instruction limit (move_matmul_waits_to_ldweights +
    # generate_event_semaphores) — walrus codegen rejects multi-wait
    # instructions otherwise.
    nc = bacc.Bacc(None)
    f8 = mybir.dt.float8e4
    f32 = mybir.dt.float32

    wpack = nc.dram_tensor("wpack", [E, P, W1_COLS + W2_COLS], f8, kind="ExternalInput")
    # pack = ft (512B) | tgt8 (512B) | -iden (128B) per partition, one
    # fat-descriptor DMA instead of three 512B-desc crawls.
    packp = nc.dram_tensor("packp", [P, 1152], f8, kind="ExternalInput")
    b1p = nc.dram_tensor("b1p", [P, E, MC], f32, kind="ExternalInput")
    b2p = nc.dram_tensor("b2p", [8, E, D], f8, kind="ExternalInput")
    # per-partition partial sums; the host adds the 128 values (and the 8
    # cores' results) — keeps the cross-partition reduce off the kernel tail.
    loss = nc.dram_tensor("loss", [P, 1], f32, kind="ExternalOutput")

    with tile.TileContext(nc) as tc:
        with (
            tc.tile_pool(name="w1pool", bufs=E) as w1pool,
            tc.tile_pool(name="w2pool", bufs=2 * E) as w2pool,
            tc.tile_pool(name="cpool", bufs=1) as cpool,
            tc.tile_pool(name="zpool", bufs=8) as zpool,
            tc.tile_pool(name="hpool", bufs=E) as hpool,
            tc.tile_pool(name="spool", bufs=2) as spool,
            tc.tile_pool(name="apool", bufs=3) as apool,
            tc.tile_pool(name="psz", bufs=4, space="PSUM") as psz,
            tc.tile_pool(name="pso", bufs=3, space="PSUM") as pso,
            tc.tile_pool(name="psf", bufs=1, space="PSUM") as psf,
        ):
            # The whole weight stream lives on the sync HWDGE ring, in
            # consumption order.  Measured dead ends: (a) HWDGE rings are
            # only ~2 DMAs deep, so a long stream holds its issuing engine
            # captive at the dma_start instructions — harmless for the idle
            # sync engine, fatal for ACT (the gelu stream starves and the
            # pipeline collapses); (b) SWDGE (gpsimd) as a second stream
            # generator serializes on descriptor emission (~2-4us per slab)
            # and dilutes aggregate rate to ~285 GB/s vs ~350 single-ring.
            w1ts, w2ats, w2bts = {}, {}, {}

            def issue_w1(e, eng):
                w1ts[e] = w1pool.tile([P, W1_COLS], f8, tag="w1", name=f"w1t{e}")
                eng.dma_start(w1ts[e][:], wpack[e][:, :W1_COLS])

            def issue_w2_half(e, half, eng, parts=1):
                t = w2pool.tile([P, MC // 4, 2, D], f8, tag="w2", name=f"w2t{e}h{half}")
                lo = W1_COLS + half * W2_HALF
                src = wpack[e][:, lo : lo + W2_HALF].rearrange(
                    "p (k two d) -> p k two d", two=2, d=D
                )
                # parts>1 splits the transfer so the PE can start on the
                # early chunks before the final byte lands (used for the
                # last-arriving half-slab only).
                kstep = (MC // 4) // parts
                for pi in range(parts):
                    eng.dma_start(
                        t[:, pi * kstep : (pi + 1) * kstep],
                        src[:, pi * kstep : (pi + 1) * kstep],
                    )
                if half == 0:
                    w2ats[e] = t
                else:
                    w2bts[e] = t

            # scalar ring: only the two phase-1 inputs (~1.5us of desc-gen up
            # front), then the ACT engine is free for tables + gelus.  b2
            # (needed ~31us) rides SWDGE so it touches neither HWDGE ring.
            pack = cpool.tile([P, 1152], f8)
            nc.scalar.dma_start(pack[:], packp[:])
            ft = pack[:, 0:512].rearrange("p (k b) -> p k b", k=KC1)
            tgt8 = pack[:, 512:1024]
            iden = pack[:, 1024:1152]
            b1s = cpool.tile([P, E, MC], f32)
            nc.scalar.dma_start(b1s[:], b1p[:])
            b2s = cpool.tile([8, E, D], f8)
            nc.gpsimd.dma_start(b2s[:], b2p[:])

            for e in range(E):
                issue_w1(e, nc.sync)
            for e in range(E):
                issue_w2_half(e, 0, nc.sync)
                issue_w2_half(e, 1, nc.sync, parts=2 if e == E - 1 else 1)

            # Warm the PE HAM clock-gate (idle PE runs at 1.2 GHz; ~3.4us of
            # sustained activity unlocks 2.4 GHz) with throwaway matmuls on a
            # zeroed tile while the first weight slab is still in flight.
            # wsrc's memset comes FIRST on the DVE queue — nothing upstream of
            # it may depend on a DMA.
            wsrc = cpool.tile([P, D], f8)
            nc.vector.memset(wsrc[:], 0.0)
            # lhsT for the +b2 fix matmul: sum_k 0.125 * b2 = b2 over K=8.
            ones8 = cpool.tile([8, P], f8)
            nc.vector.memset(ones8[:], 0.125)
            # Advance the DVE vector clock past the b1s DMA with a one-element
            # read so the bias-add TTs only need their PE wait.
            dummy = cpool.tile([1, 2], f32)
            nc.vector.tensor_copy(dummy[:, 0:1], b1s[:1, 0, :1])

            pwarm = psf.tile([P, D], mybir.dt.float32, tag="warm")
            NWARM = 8
            for i in range(NWARM):
                nc.tensor.matmul(
                    pwarm[:], lhsT=wsrc[:, :P], rhs=wsrc[:],
                    start=(i == 0), stop=(i == NWARM - 1),
                )

            # Phase 1: layer-1 + gelu for ALL experts. PE executes its queue
            # in program order, so trailing layer-2 work must not sit between
            # layer-1 passes — this way the last expert's bias/gelu chain
            # drains while layer-2 matmuls for earlier experts run, instead of
            # serializing at the end of the kernel.
            hsbs = {}
            for e in range(E):
                w1v = w1ts[e][:].rearrange("p (k h) -> p k h", k=KC1)
                hsb = hpool.tile([P, MC, P], f8, tag="h", name=f"hsb{e}")
                hsbs[e] = hsb
                for g in range(NG):
                    zp = psz.tile([P, NG, P], mybir.dt.float32, tag="zp")
                    for mc in range(NG):
                        m = g * NG + mc
                        # fp8 DoubleRow on layer 1: contract two 128-row
                        # D-chunks per matmul (half the instruction count; the
                        # exposed per-matmul LDWEIGHTS cost is what limits
                        # layer 1, since N=B=128 is short).
                        for kc in range(KC1 // 2):
                            nc.tensor.matmul(
                                zp[:, mc],
                                lhsT=w1v[:, 2 * kc : 2 * kc + 2, m * P : (m + 1) * P],
                                rhs=ft[:, 2 * kc : 2 * kc + 2, :],
                                start=(kc == 0),
                                stop=(kc == KC1 // 2 - 1),
                                perf_mode=mybir.MatmulPerfMode.DoubleRow,
                            )
                    zb = zpool.tile([P, NG, P], mybir.dt.bfloat16, tag="zb")
                    nc.vector.tensor_tensor(
                        zb[:],
                        zp[:],
                        b1s[:, e, g * NG : (g + 1) * NG, None].to_broadcast([P, NG, P]),
                        mybir.AluOpType.add,
                    )
                    nc.scalar.activation(
                        hsb[:, g * NG : (g + 1) * NG],
                        zb[:],
                        act_fn,
                    )

            # Phase 2: layer-2 + loss accumulation for all experts.
            acc = None
            for e in range(E):
                hsb = hsbs[e]
                po = pso.tile([P, D], mybir.dt.float32, tag="po")
                # fp8 DoubleRow: each matmul contracts a pair of 128-row
                # k-chunks (array virtualized to 256 rows).  kc 0..3 read the
                # first W2 half-slab, kc 4..7 the second, so the first half of
                # each expert's chain starts half a slab early.  The -tgt/+b2
                # fix matmuls sit BETWEEN the halves: after the last W2 byte
                # lands, only 4 DR matmuls remain before the Square.
                for kc in range(MC // 4):
                    nc.tensor.matmul(
                        po[:],
                        lhsT=hsb[:, 2 * kc : 2 * kc + 2, :],
                        rhs=w2ats[e][:, kc],
                        start=(kc == 0),
                        stop=False,
                        perf_mode=mybir.MatmulPerfMode.DoubleRow,
                    )
                # Fold -target and +b2 into the same PSUM accumulation:
                # (-I) @ tgt subtracts the target; (0.125-ones [8,128]).T @
                # (8x-replicated b2) adds the bias.  err then sits in PSUM.
                nc.tensor.matmul(
                    po[:], lhsT=iden, rhs=tgt8, start=False, stop=False,
                )
                nc.tensor.matmul(
                    po[:], lhsT=ones8[:], rhs=b2s[:, e], start=False, stop=False,
                )
                for kc in range(MC // 4, MC // 2):
                    nc.tensor.matmul(
                        po[:],
                        lhsT=hsb[:, 2 * kc : 2 * kc + 2, :],
                        rhs=w2bts[e][:, kc - MC // 4],
                        start=False,
                        stop=(kc == MC // 2 - 1),
                        perf_mode=mybir.MatmulPerfMode.DoubleRow,
                    )

                # square + row-sum in one ACT pass straight from PSUM (fp32
                # accumulator); the Square output itself is scrap.
                sq = spool.tile([P, D], mybir.dt.bfloat16, tag="sq")
                red = apool.tile([P, 1], mybir.dt.float32, tag="red")
                nc.scalar.activation(
                    sq[:], po[:], mybir.ActivationFunctionType.Square,
                    accum_out=red[:],
                )
                nacc = apool.tile([P, 1], mybir.dt.float32, tag="acc")
                if acc is None:
                    nc.vector.tensor_copy(nacc[:], red[:])
                else:
                    nc.vector.tensor_add(nacc[:], acc[:], red[:])
                acc = nacc

            nc.sync.dma_start(loss[:], acc[:])

    nc.finalize()
    return nc


def get_nc(act="gelu"):
    global _NC
    if _NC is None:
        _NC = _build_nc(act)
    return _NC


def make_in_maps(features, target_features, W1, b1, W2, b2):
    features = np.asarray(features, np.float32)
    target_features = np.asarray(target_features, np.float32)
    W1 = np.asarray(W1, np.float32)
    b1 = np.asarray(b1, np.float32)
    W2 = np.asarray(W2, np.float32)
    b2 = np.asarray(b2, np.float32)

    # pack weights partition-major: wpack[a][e][p, kc*H + col] = W1[a,e,kc*128+p,col]
    #                              wpack[a][e][p, 8192 + kc*D + d] = W2[a,e,kc*128+p,d]
    w1p = np.ascontiguousarray(
        W1.reshape(E, E, KC1, P, H).transpose(0, 1, 3, 2, 4).reshape(E, E, P, W1_COLS)
    ).astype(F8)
    w2p = np.ascontiguousarray(
        W2.reshape(E, E, MC, P, D).transpose(0, 1, 3, 2, 4).reshape(E, E, P, W2_COLS)
    ).astype(F8)
    wpk = np.concatenate([w1p, w2p], axis=3)  # [A, E, P, 16384] fp8

    iden = (-np.eye(P, dtype=np.float32)).astype(F8)
    in_maps = []
    for a in range(E):
        fa = features[:, a]  # [B, D]
        ftp = np.ascontiguousarray(
            fa.T.reshape(KC1, P, B).transpose(1, 0, 2)
        ).astype(F8).reshape(P, KC1 * B)
        tgtp = np.ascontiguousarray(target_features[:, a]).astype(F8)  # [B, D]
        packp = np.concatenate([ftp, tgtp, iden], axis=1)  # [P, 1152] fp8
        b1pa = np.ascontiguousarray(b1[a].reshape(E, MC, P).transpose(2, 0, 1))  # [P,E,MC]
        b2pa = np.ascontiguousarray(
            np.broadcast_to(b2[a][None], (8, E, D))
        ).astype(F8)  # [8, E, D]
        in_maps.append(
            {
                "wpack": wpk[a],
                "packp": packp,
                "b1p": b1pa,
                "b2p": b2pa,
            }
        )
    return in_maps


def kernel(features, target_features, W1, b1, W2, b2):
    from concourse.bass_utils import run_bass_kernel_spmd

    nc = get_nc()
    in_maps = make_in_maps(features, target_features, W1, b1, W2, b2)
    res = run_bass_kernel_spmd(nc, in_maps, list(range(E)))
    total = sum(float(np.asarray(r["loss"], np.float64).sum()) for r in res.results)
    return np.float32(total / (B * D * E))


# revision 30
# speedup vs baseline: 1.2218x; 1.2218x over previous
"""Trainium2 Bass kernel for nn_AveragedAdapter (dense_mlp).

Computes: loss = sum_{a,e} mean_{b,d} (gelu(f[:,a] @ W1[a,e] + b1[a,e]) @ W2[a,e]
                                        + b2[a,e] - target[:,a])^2 / E

Sharding: expert-parallel over the first expert axis `a` — core a computes the
full inner-e loop for its adapter row and returns a partial sum of squared
errors; the host adds the 8 partials and applies the 1/(B*D*E) scale.

The 16.8 MiB/core of fp8 weights dominate: the kernel is one long HBM->SBUF
stream over the 16 SDMA engines (~26.5 GB/s each), so exec time ~=
stream_start + per-engine bytes / rate + tail.  Design choices follow:

  - Weights fp8-e4m3 (loss is a mean over 33.5M squared errors; measured
    rel-err ~1e-3 end to end).  Biases and accumulation stay fp32.
  - W1[0] rides the scalar HWDGE ring (whose path is warmed by the ACT
    table static DMAs and starts ~2.5us before the sync ring's first
    byte); ft/b1 ride scalar ahead of it.  All other slabs stream on the
    sync ring in consumption order: W1[1..7], then W2 halves.
  - W2 slabs are split into halves so the last expert's first 4 matmuls
    start half a slab before the final byte lands.
  - The per-expert (target - b2) tensor is NOT streamed (it was 1 MiB of
    bf16).  Instead target[:,a] is loaded once as fp8 [128,512] and folded
    into the layer-2 PSUM accumulation with a -identity matmul; b2 is
    added with a [8,128] 0.125-ones x [8,512] 8x-replicated-b2 matmul.
    The Square+row-sum then reads PSUM directly (no DVE subtract, no err
    SBUF roundtrip) — the only DVE work left in phase 2 is the [128,1]
    running-sum chain.
  - Late-needed consts (tgt8, -I, b2) ride the gpsimd SWDGE ring so they
    never touch the HWDGE streams.
  - Throwaway matmuls warm the PE clock-gate during the first slab's
    flight (idle PE runs at 1.2 GHz; sustained activity unlocks 2.4 GHz).

Per-core program (a = core id):
  - phase 1 (all experts): layer 1 computes hT (H on partitions) with W1
    chunk-pairs stationary via fp8 DoubleRow ([128,2,128] x [128,2,128]),
    4 m-chunks per PSUM bank; bias add on DVE (broadcast over batch) -> bf16;
    exact-erf Gelu on ACT -> fp8 h kept in SBUF for all 8 experts.
  - phase 2 (all experts): layer 2 with h chunk-pairs stationary, W2 moving
    [128,2,512] DoubleRow, 8 matmuls + the -tgt and +b2 fix matmuls into one
    PSUM bank; Square+row-sum on ACT straight from PSUM (accum_out);
    [128,1] running sum chained on DVE; final cross-partition sum via a
    ones-vector matmul -> [1,1] -> DMA out.
"""

import sys

if "/opt/trn_rl_repo" not in sys.path:
    sys.path.insert(0, "/opt/trn_rl_repo")

import numpy as np
import ml_dtypes

B, E, D, M = 128, 8, 512, 4
H = M * D            # 2048
P = 128
KC1 = D // P         # 4  k-chunks in layer 1
MC = H // P          # 16 m-chunks of H / k-chunks in layer 2
NG = 4               # m-chunk groups (4 chunks -> one PSUM bank)
W1_COLS = KC1 * H    # 8192
W2_COLS = MC * D     # 8192
W2_HALF = W2_COLS // 2
F8 = ml_dtypes.float8_e4m3

_NC = None


def _build_nc(act="gelu"):
    import concourse.tile as tile
    from concourse import bacc, mybir

    act_fn = {
        "gelu": mybir.ActivationFunctionType.Gelu,
        "identity": mybir.ActivationFunctionType.Identity,
    }[act]
    # Bacc (not Bass): its compile() pass legalizes sync waits for the trn2
    # ISA's one-wait-per-instruction limit (move_matmul_waits_to_ldweights +
    # generate_event_semaphores) — walrus codegen rejects multi-wait
    # instructions otherwise.
    nc = bacc.Bacc(None)
    f8 = mybir.dt.float8e4
    f32 = mybir.dt.float32

    wpack = nc.dram_tensor("wpack", [E, P, W1_COLS + W2_COLS], f8, kind="ExternalInput")
    # pack = ft (512B) | tgt8 (512B) | -iden (128B) per partition, one
    # fat-descriptor DMA instead of three 512B-desc crawls.
    packp = nc.dram_tensor("packp", [P, 1152], f8, kind="ExternalInput")
    b1p = nc.dram_tensor("b1p", [P, E, MC], f32, kind="ExternalInput")
    b2p = nc.dram_tensor("b2p", [8, E, D], f8, kind="ExternalInput")
    loss = nc.dram_tensor("loss", [1, 1], f32, kind="ExternalOutput")

    with tile.TileContext(nc) as tc:
        with (
            tc.tile_pool(name="w1pool", bufs=E) as w1pool,
            tc.tile_pool(name="w2pool", bufs=2 * E) as w2pool,
            tc.tile_pool(name="cpool", bufs=1) as cpool,
            tc.tile_pool(name="zpool", bufs=8) as zpool,
            tc.tile_pool(name="hpool", bufs=E) as hpool,
            tc.tile_pool(name="spool", bufs=2) as spool,
            tc.tile_pool(name="apool", bufs=3) as apool,
            tc.tile_pool(name="psz", bufs=4, space="PSUM") as psz,
            tc.tile_pool(name="pso", bufs=3, space="PSUM") as pso,
            tc.tile_pool(name="psf", bufs=1, space="PSUM") as psf,
        ):
            # The whole weight stream lives on the sync HWDGE ring, in
            # consumption order.  Measured dead ends: (a) HWDGE rings are
            # only ~2 DMAs deep, so a long stream holds its issuing engine
            # captive at the dma_start instructions — harmless for the idle
            # sync engine, fatal for ACT (the gelu stream starves and the
            # pipeline collapses); (b) SWDGE (gpsimd) as a second stream
            # generator serializes on descriptor emission (~2-4us per slab)
            # and dilutes aggregate rate to ~285 GB/s vs ~350 single-ring.
            w1ts, w2ats, w2bts = {}, {}, {}

            def issue_w1(e, eng):
                w1ts[e] = w1pool.tile([P, W1_COLS], f8, tag="w1", name=f"w1t{e}")
                eng.dma_start(w1ts[e][:], wpack[e][:, :W1_COLS])

            def issue_w2_half(e, half, eng, parts=1):
                t = w2pool.tile([P, MC // 4, 2, D], f8, tag="w2", name=f"w2t{e}h{half}")
                lo = W1_COLS + half * W2_HALF
                src = wpack[e][:, lo : lo + W2_HALF].rearrange(
                    "p (k two d) -> p k two d", two=2, d=D
                )
                # parts>1 splits the transfer so the PE can start on the
                # early chunks before the final byte lands (used for the
                # last-arriving half-slab only).
                kstep = (MC // 4) // parts
                for pi in range(parts):
                    eng.dma_start(
                        t[:, pi * kstep : (pi + 1) * kstep],
                        src[:, pi * kstep : (pi + 1) * kstep],
                    )
                if half == 0:
                    w2ats[e] = t
                else:
                    w2bts[e] = t

            # scalar ring: only the two phase-1 inputs (~1.5us of desc-gen up
            # front), then the ACT engine is free for tables + gelus.  b2
            # (needed ~31us) rides SWDGE so it touches neither HWDGE ring.
            pack = cpool.tile([P, 1152], f8)
            nc.scalar.dma_start(pack[:], packp[:])
            ft = pack[:, 0:512].rearrange("p (k b) -> p k b", k=KC1)
            tgt8 = pack[:, 512:1024]
            iden = pack[:, 1024:1152]
            b1s = cpool.tile([P, E, MC], f32)
            nc.scalar.dma_start(b1s[:], b1p[:])
            b2s = cpool.tile([8, E, D], f8)
            nc.gpsimd.dma_start(b2s[:], b2p[:])

            for e in range(E):
                issue_w1(e, nc.sync)
            for e in range(E):
                issue_w2_half(e, 0, nc.sync)
                issue_w2_half(e, 1, nc.sync)

            # Warm the PE HAM clock-gate (idle PE runs at 1.2 GHz; ~3.4us of
            # sustained activity unlocks 2.4 GHz) with throwaway matmuls on a
            # zeroed tile while the first weight slab is still in flight.
            # wsrc's memset comes FIRST on the DVE queue — nothing upstream of
            # it may depend on a DMA.
            wsrc = cpool.tile([P, D], f8)
            nc.vector.memset(wsrc[:], 0.0)
            ones = cpool.tile([P, 1], f32)
            nc.vector.memset(ones[:], 1.0)
            # lhsT for the +b2 fix matmul: sum_k 0.125 * b2 = b2 over K=8.
            ones8 = cpool.tile([8, P], f8)
            nc.vector.memset(ones8[:], 0.125)
            # Advance the DVE vector clock past the b1s DMA with a one-element
            # read so the bias-add TTs only need their PE wait.
            dummy = cpool.tile([1, 2], f32)
            nc.vector.tensor_copy(dummy[:, 0:1], b1s[:1, 0, :1])

            pwarm = psf.tile([P, D], mybir.dt.float32, tag="warm")
            NWARM = 8
            for i in range(NWARM):
                nc.tensor.matmul(
                    pwarm[:], lhsT=wsrc[:, :P], rhs=wsrc[:],
                    start=(i == 0), stop=(i == NWARM - 1),
                )

            # Phase 1: layer-1 + gelu for ALL experts. PE executes its queue
            # in program order, so trailing layer-2 work must not sit between
            # layer-1 passes — this way the last expert's bias/gelu chain
            # drains while layer-2 matmuls for earlier experts run, instead of
            # serializing at the end of the kernel.
            hsbs = {}
            for e in range(E):
                w1v = w1ts[e][:].rearrange("p (k h) -> p k h", k=KC1)
                hsb = hpool.tile([P, MC, P], f8, tag="h", name=f"hsb{e}")
                hsbs[e] = hsb
                for g in range(NG):
                    zp = psz.tile([P, NG, P], mybir.dt.float32, tag="zp")
                    for mc in range(NG):
                        m = g * NG + mc
                        # fp8 DoubleRow on layer 1: contract two 128-row
                        # D-chunks per matmul (half the instruction count; the
                        # exposed per-matmul LDWEIGHTS cost is what limits
                        # layer 1, since N=B=128 is short).
                        for kc in range(KC1 // 2):
                            nc.tensor.matmul(
                                zp[:, mc],
                                lhsT=w1v[:, 2 * kc : 2 * kc + 2, m * P : (m + 1) * P],
                                rhs=ft[:, 2 * kc : 2 * kc + 2, :],
                                start=(kc == 0),
                                stop=(kc == KC1 // 2 - 1),
                                perf_mode=mybir.MatmulPerfMode.DoubleRow,
                            )
                    zb = zpool.tile([P, NG, P], mybir.dt.bfloat16, tag="zb")
                    nc.vector.tensor_tensor(
                        zb[:],
                        zp[:],
                        b1s[:, e, g * NG : (g + 1) * NG, None].to_broadcast([P, NG, P]),
                        mybir.AluOpType.add,
                    )
                    nc.scalar.activation(
                        hsb[:, g * NG : (g + 1) * NG],
                        zb[:],
                        act_fn,
                    )

            # Phase 2: layer-2 + loss accumulation for all experts.
            acc = None
            for e in range(E):
                hsb = hsbs[e]
                po = pso.tile([P, D], mybir.dt.float32, tag="po")
                # fp8 DoubleRow: each matmul contracts a pair of 128-row
                # k-chunks (array virtualized to 256 rows).  kc 0..3 read the
                # first W2 half-slab, kc 4..7 the second, so the first half of
                # each expert's chain starts half a slab early.  The -tgt/+b2
                # fix matmuls sit BETWEEN the halves: after the last W2 byte
                # lands, only 4 DR matmuls remain before the Square.
                for kc in range(MC // 4):
                    nc.tensor.matmul(
                        po[:],
                        lhsT=hsb[:, 2 * kc : 2 * kc + 2, :],
                        rhs=w2ats[e][:, kc],
                        start=(kc == 0),
                        stop=False,
                        perf_mode=mybir.MatmulPerfMode.DoubleRow,
                    )
                # Fold -target and +b2 into the same PSUM accumulation:
                # (-I) @ tgt subtracts the target; (0.125-ones [8,128]).T @
                # (8x-replicated b2) adds the bias.  err then sits in PSUM.
                nc.tensor.matmul(
                    po[:], lhsT=iden, rhs=tgt8, start=False, stop=False,
                )
                nc.tensor.matmul(
                    po[:], lhsT=ones8[:], rhs=b2s[:, e], start=False, stop=False,
                )
                for kc in range(MC // 4, MC // 2):
                    nc.tensor.matmul(
                        po[:],
                        lhsT=hsb[:, 2 * kc : 2 * kc + 2, :],
                        rhs=w2bts[e][:, kc - MC // 4],
                        start=False,
                        stop=(kc == MC // 2 - 1),
                        perf_mode=mybir.MatmulPerfMode.DoubleRow,
                    )

                # square + row-sum in one ACT pass straight from PSUM (fp32
                # accumulator); the Square output itself is scrap.
                sq = spool.tile([P, D], mybir.dt.bfloat16, tag="sq")
                red = apool.tile([P, 1], mybir.dt.float32, tag="red")
                nc.scalar.activation(
                    sq[:], po[:], mybir.ActivationFunctionType.Square,
                    accum_out=red[:],
                )
                nacc = apool.tile([P, 1], mybir.dt.float32, tag="acc")
                if acc is None:
                    nc.vector.tensor_copy(nacc[:], red[:])
                else:
                    nc.vector.tensor_add(nacc[:], acc[:], red[:])
                acc = nacc

            pf = psf.tile([1, 1], mybir.dt.float32, tag="warm")
            nc.tensor.matmul(pf[:], lhsT=ones[:], rhs=acc[:], start=True, stop=True)
            osb = cpool.tile([1, 1], mybir.dt.float32)
            nc.vector.tensor_copy(osb[:], pf[:])
            nc.sync.dma_start(loss[:], osb[:])

    nc.finalize()
    return nc


def get_nc(act="gelu"):
    global _NC
    if _NC is None:
        _NC = _build_nc(act)
    return _NC


def make_in_maps(features, target_features, W1, b1, W2, b2):
    features = np.asarray(features, np.float32)
    target_features = np.asarray(target_features, np.float32)
    W1 = np.asarray(W1, np.float32)
    b1 = np.asarray(b1, np.float32)
    W2 = np.asarray(W2, np.float32)
    b2 = np.asarray(b2, np.float32)

    # pack weights partition-major: wpack[a][e][p, kc*H + col] = W1[a,e,kc*128+p,col]
    #                              wpack[a][e][p, 8192 + kc*D + d] = W2[a,e,kc*128+p,d]
    w1p = np.ascontiguousarray(
        W1.reshape(E, E, KC1, P, H).transpose(0, 1, 3, 2, 4).reshape(E, E, P, W1_COLS)
    ).astype(F8)
    w2p = np.ascontiguousarray(
        W2.reshape(E, E, MC, P, D).transpose(0, 1, 3, 2, 4).reshape(E, E, P, W2_COLS)
    ).astype(F8)
    wpk = np.concatenate([w1p, w2p], axis=3)  # [A, E, P, 16384] fp8

    iden = (-np.eye(P, dtype=np.float32)).astype(F8)
    in_maps = []
    for a in range(E):
        fa = features[:, a]  # [B, D]
        ftp = np.ascontiguousarray(
            fa.T.reshape(KC1, P, B).transpose(1, 0, 2)
        ).astype(F8).reshape(P, KC1 * B)
        tgtp = np.ascontiguousarray(target_features[:, a]).astype(F8)  # [B, D]
        packp = np.concatenate([ftp, tgtp, iden], axis=1)  # [P, 1152] fp8
        b1pa = np.ascontiguousarray(b1[a].reshape(E, MC, P).transpose(2, 0, 1))  # [P,E,MC]
        b2pa = np.ascontiguousarray(
            np.broadcast_to(b2[a][None], (8, E, D))
        ).astype(F8)  # [8, E, D]
        in_maps.append(
            {
                "wpack": wpk[a],
                "packp": packp,
                "b1p": b1pa,
                "b2p": b2pa,
            }
        )
    return in_maps


def kernel(features, target_features, W1, b1, W2, b2):
    from concourse.bass_utils import run_bass_kernel_spmd

    nc = get_nc()
    in_maps = make_in_maps(features, target_features, W1, b1, W2, b2)
    res = run_bass_kernel_spmd(nc, in_maps, list(range(E)))
    total = sum(float(np.asarray(r["loss"], np.float64).sum()) for r in res.results)
    return np.float32(total / (B * D * E))
